# revision 5
# baseline (speedup 1.0000x reference)
"""Trainium2 Bass kernel for the quirky MultiHeadAttention module (v2).

Reference computation (S = D = 4096, 16 "heads" that are chunks of 256 ROWS):
    q = x @ Wq.T + bq ; k = x @ Wk.T + bk ; v = x @ Wv.T + bv
    per head h (rows h*256..h*256+255):
        scores = split(v)_h @ split(k)_h.T / 64 ; attn = softmax(scores, -1)
        out_h  = attn @ split(q)_h
    result = concat(out_h) @ Wo.T + bo

Sharding: pure data-parallel over token rows. Each of the 8 cores owns 512
rows = exactly 2 complete "heads"; every stage is row-local given full
weights, so no collectives.

Changes vs the 994us v1 baseline:
  - all-fp16 operands (v1 used bf16 for q/Wo stages + a second bf16 copy of
    x). One resident fp16 xT serves the k/v projections (moving operand) AND
    the q projection (stationary operand): -4MB SBUF, -4MB DMA, and the
    q-projection phase no longer waits on a second activation load.
  - single long-lived pool set; PSUM is statically budgeted to 8 banks
    (acc ring 5 x [128,512] + attention ring 3 x [128,256]) so no phase
    transition closes/reopens pools (v1 lost ~10us + HAM re-throttle to
    pool-boundary barriers).
  - softmax normalization chain (Z ones-matmul -> DVE reciprocal -> f16
    broadcast matmul -> DVE multiply) is interleaved into the q-projection
    phase with all consumers scheduled after their producers have ~30us of
    slack: zero PE stalls (v1 serialized the fp32r broadcast behind a 2us
    DVE reciprocal in the middle of the S.T matmul stream).
  - no fp32r / no GpSimd memset / no bf16 anywhere: uniform fp16 keeps the
    NEFF free of cast-DMA features and the whole PE stream at 1 column/cycle
    (216ns per 512-col matmul at 2.37GHz; the device clock itself floats
    between ~1.98 and ~2.37GHz per run for reasons outside kernel control).
  - tail: the last output chunk folds bo in on the PE (rank-1 matmul) so its
    PSUM drains are pure copies split across ACT+DVE, and output DMAs
    alternate between both hwdge queues -- the post-matmul tail is ~5us of
    drain+DMA plus the fixed ~8us framework epilogue.
  - fractional fp8: the last 4 of 32 k-blocks of the final projection run as
    e4m3 DoubleRow matmuls (2 k-blocks per instruction at 2x rate, -14us PE).
    The whole final stage is scaled x64 (E.T normalized by 8/Z so OT carries
    x8; Wo slabs x8 in fp16 / x8-then-e4m3 in fp8) so the fp16 and fp8-DR
    terms share one PSUM accumulation group; drains rescale by 1/64 in the
    ACT/DVE copy. Measured end-to-end error 1.33e-2 of absmax (budget 2e-2;
    numpy simulation of the same quantization predicts 1.31e-2).
  - startup: first matmul only needs slab(k,0) quarter 0 + xT[:,0,:]
    (~384KB) instead of the full 7MB prefetch (v1 idled the PE ~25us).

Per-core dataflow (phases in PE program order):
  B/C:  kT = (x@Wk.T+bk).T, vT likewise -> resident SBUF [128, kb, tok] f16,
        bias folded into the ACT Identity drain (per-partition bias).
  S.T:  S.T[j,i] = sum_d k[j,d] v[i,d] accumulated in PSUM per head,
        E.T = exp(S.T/64) f16 (no max-subtraction: |logits| <~ 8).
  A:    qn = x@Wq.T + bq natural [tok, feat] f16 (stationary = xT blocks,
        moving = Wq slabs); Z/recip/bcast/normalize of E.T run concurrently.
  O.T:  O.T[d,i] = sum_j qn[j,d] E.T_norm[j,i] -> OT f16 (overlays kT).
  final: out rows = OT.T @ Wo.T + bo, f32 to DRAM.
"""

import numpy as np

import concourse.bass as bass
import concourse.bacc as bacc
import concourse.mybir as mybir
import concourse.tile as tile
from concourse.bass_utils import run_bass_kernel_spmd

F32 = mybir.dt.float32
F16 = mybir.dt.float16
F8 = mybir.dt.float8e4
DR = mybir.MatmulPerfMode.DoubleRow
U8 = mybir.dt.uint8
NF8 = 4  # final-stage k-blocks computed in fp8-DoubleRow
AF = mybir.ActivationFunctionType

D = 4096          # d_model == seq
NCORE = 8
SH = D // NCORE   # 512 token rows per core
KB = D // 128     # 32 contraction blocks of 128
NO = D // 512     # 8 output-feature chunks of 512
SM = SH // 128    # 4 token blocks of 128 per core
SCALE = 1.0 / 64.0  # 1/sqrt(4096)


def _build():
    nc = bacc.Bacc(
        "TRN2",
        target_bir_lowering=False,
        debug=False,
        enable_asserts=False,
        num_devices=NCORE,
    )

    xTp = nc.declare_dram_parameter("xTp", [128, KB, SH], F16, isOutput=False)
    wkp = nc.declare_dram_parameter("wkp", [KB, 128, KB, 128], F16, isOutput=False)
    wvp = nc.declare_dram_parameter("wvp", [KB, 128, KB, 128], F16, isOutput=False)
    wqp = nc.declare_dram_parameter("wqp", [NO, KB, 128, 512], F16, isOutput=False)
    wop = nc.declare_dram_parameter("wop", [NO, KB, 128, 512], F16, isOutput=False)
    wo8p = nc.declare_dram_parameter("wo8p", [NO, NF8 // 2, 128, 2, 512], U8, isOutput=False)
    bob8 = nc.declare_dram_parameter("bob8", [1, D], F16, isOutput=False)
    bk_p = nc.declare_dram_parameter("bk_p", [128, KB], F32, isOutput=False)
    bv_p = nc.declare_dram_parameter("bv_p", [128, KB], F32, isOutput=False)
    bqb = nc.declare_dram_parameter("bqb", [128, D], F16, isOutput=False)
    bob = nc.declare_dram_parameter("bob", [128, D], F16, isOutput=False)
    ones_c = nc.declare_dram_parameter("ones_c", [128, 1], F16, isOutput=False)
    ones_r = nc.declare_dram_parameter("ones_r", [1, 128], F16, isOutput=False)
    zero_c = nc.declare_dram_parameter("zero_c", [128, 1], F32, isOutput=False)
    out = nc.declare_dram_parameter("out", [SH, D], F32, isOutput=True)

    with tile.TileContext(nc) as tc:
        with (
            nc.allow_low_precision(reason="fp16 matmul operands, fp32 accumulate"),
            tc.tile_pool(name="cpool", bufs=1) as cpool,
            tc.tile_pool(name="xpool", bufs=1) as xpool,
            tc.tile_pool(name="kvq", bufs=1) as kvq,
            tc.tile_pool(name="big", bufs=2) as bigp,
            tc.tile_pool(name="wslab", bufs=3) as wslab,
            tc.tile_pool(name="wa", bufs=12) as wa,
            tc.tile_pool(name="etp", bufs=1) as etp,
            tc.tile_pool(name="stf", bufs=4) as stf,
            tc.tile_pool(name="psacc", bufs=5, space="PSUM") as psacc,
            tc.tile_pool(name="psatt", bufs=3, space="PSUM") as psatt,
        ):
            # ---- startup DMAs ----
            # The x load (4MB) gates the whole first projection chain; split
            # it across BOTH hwdge queues (Sync + Scalar) in 4-block granules
            # interleaved with the first weight slab's quarters so chain m=0
            # streams as data lands.
            xT = xpool.tile([128, KB, SH], F16, name="xT")
            slab0 = wslab.tile([128, KB, 128], F16, tag="slab", name="slab_k_0")
            nc.sync.dma_start(slab0[:, 0:2, :], wkp[0][:, 0:2, :])
            nc.scalar.dma_start(xT[:, 0:2, :], xTp[:, 0:2, :])
            nc.sync.dma_start(slab0[:, 2:8, :], wkp[0][:, 2:8, :])
            nc.scalar.dma_start(xT[:, 2:4, :], xTp[:, 2:4, :])
            nc.sync.dma_start(xT[:, 4:8, :], xTp[:, 4:8, :])
            nc.scalar.dma_start(xT[:, 8:12, :], xTp[:, 8:12, :])
            nc.sync.dma_start(
                slab0[:, 8:16, :], wkp[0][:, 8:16, :]
            )
            nc.scalar.dma_start(xT[:, 16:20, :], xTp[:, 16:20, :])
            nc.sync.dma_start(xT[:, 12:16, :], xTp[:, 12:16, :])
            nc.sync.dma_start(slab0[:, 16:24, :], wkp[0][:, 16:24, :])
            nc.scalar.dma_start(xT[:, 24:28, :], xTp[:, 24:28, :])
            nc.sync.dma_start(xT[:, 20:24, :], xTp[:, 20:24, :])
            nc.sync.dma_start(slab0[:, 24:32, :], wkp[0][:, 24:32, :])
            nc.scalar.dma_start(xT[:, 28:32, :], xTp[:, 28:32, :])
            pre = [slab0]
            for m in (1, 2):
                s = wslab.tile([128, KB, 128], F16, tag="slab", name=f"slab_k_{m}")
                nc.sync.dma_start(s[:], wkp[m][:])
                pre.append(s)
            bkv = cpool.tile([128, 2 * KB], F32, name="bkv")
            nc.scalar.dma_start(bkv[:, 0:KB], bk_p[:])
            nc.scalar.dma_start(bkv[:, KB : 2 * KB], bv_p[:])
            ones_col = cpool.tile([128, 1], F16, name="ones_col")
            nc.scalar.dma_start(ones_col[:], ones_c[:])
            ones_row = cpool.tile([1, 128], F16, name="ones_row")
            nc.scalar.dma_start(ones_row[:], ones_r[:])
            zero_col = cpool.tile([128, 1], F32, name="zero_col")
            nc.scalar.dma_start(zero_col[:], zero_c[:])
            bq_t = cpool.tile([128, D], F16, name="bq_t")
            nc.scalar.dma_start(bq_t[:], bqb[:])
            bo_t = cpool.tile([128, D], F16, name="bo_t")
            nc.scalar.dma_start(bo_t[:], bob[:])
            bo8_t = cpool.tile([1, D], F16, name="bo8_t")
            nc.scalar.dma_start(bo8_t[:], bob8[:])

            # kT/vT in a 2-deep ring; OT later overlays kT's slot.
            kT = bigp.tile([128, KB, SH], F16, tag="big", name="kT")
            vT = bigp.tile([128, KB, SH], F16, tag="big", name="vT")
            qn = kvq.tile([128, SM, D], F16, name="qn")
            OT8 = kvq.tile([128, NF8, SH], F8, name="OT8")

            # ---------------- B/C: kT and vT ----------------
            for which, (wp, dst) in enumerate(((wkp, kT), (wvp, vT))):
                for m in range(KB):
                    if which == 0 and m < 3:
                        slab = pre[m]
                    else:
                        slab = wslab.tile(
                            [128, KB, 128], F16, tag="slab", name=f"slab_{which}_{m}"
                        )
                        nc.sync.dma_start(slab[:], wp[m][:])
                    ps = psacc.tile(
                        [128, SH], F32, tag="acc", name=f"pskv_{which}_{m}"
                    )
                    for kb in range(KB):
                        nc.tensor.matmul(
                            ps[:],
                            slab[:, kb, :],
                            xT[:, kb, :],
                            start=(kb == 0),
                            stop=(kb == KB - 1),
                        )
                    nc.scalar.activation(
                        dst[:, m, :], ps[:], AF.Identity,
                        bias=bkv[:, which * KB + m : which * KB + m + 1],
                    )

            # ---------------- S.T + exp per head ----------------
            ets = {}
            for h in range(2):
                psS = [
                    psatt.tile([128, 256], F32, tag="att", name=f"psS_{h}_{jb}")
                    for jb in range(2)
                ]
                for kb in range(KB):
                    for jb in range(2):
                        nc.tensor.matmul(
                            psS[jb][:],
                            kT[:, kb,
                               h * 256 + jb * 128 : h * 256 + (jb + 1) * 128],
                            vT[:, kb, h * 256 : (h + 1) * 256],
                            start=(kb == 0),
                            stop=(kb == KB - 1),
                        )
                et = etp.tile([128, 2, 256], F16, name=f"et_{h}")
                ets[h] = et
                for jb in range(2):
                    nc.scalar.activation(
                        et[:, jb, :], psS[jb][:], AF.Exp,
                        bias=zero_col[:], scale=SCALE,
                    )

            # Z for h0 (PE), then its reciprocal chain on DVE runs under A n=0
            zts, zinv16s = {}, {}

            def z_matmul(h):
                zt = psatt.tile([128, 256], F32, tag="att", name=f"zt_{h}")
                zts[h] = zt
                for jb in range(2):
                    nc.tensor.matmul(
                        zt[0:1, :], ones_col[:], ets[h][:, jb, :],
                        start=(jb == 0), stop=(jb == 1),
                    )

            def z_recip(h):
                zinv32 = etp.tile([1, 256], F32, name=f"zinv32_{h}")
                nc.vector.reciprocal(zinv32[:], zts[h][0:1, :])
                zinv16 = etp.tile([1, 256], F16, name=f"zinv16_{h}")
                zinv16s[h] = zinv16
                nc.vector.tensor_copy(zinv16[:], zinv32[:])

            def bcast_norm(h):
                pb = psatt.tile([128, 256], F32, tag="att", name=f"pb_{h}")
                nc.tensor.matmul(pb[:], ones_row[:], zinv16s[h][:],
                                 start=True, stop=True)
                for jb in range(2):
                    nc.vector.tensor_mul(ets[h][:, jb, :], ets[h][:, jb, :], pb[:])

            z_matmul(0)
            z_recip(0)

            # ---------------- A: qn = x@Wq.T + bq ----------------
            for n in range(NO):
                pss = [
                    psacc.tile([128, 512], F32, tag="acc", name=f"psq_{n}_{m}")
                    for m in range(SM)
                ]
                for kb in range(KB):
                    wt = wa.tile([128, 512], F16, tag="wa", name=f"waq_{n}_{kb}")
                    nc.sync.dma_start(wt[:], wqp[n, kb][:])
                    for m in range(SM):
                        nc.tensor.matmul(
                            pss[m][:],
                            xT[:, kb, m * 128 : (m + 1) * 128],
                            wt[:],
                            start=(kb == 0),
                            stop=(kb == KB - 1),
                        )
                for m in range(SM):
                    nc.vector.tensor_add(
                        qn[:, m, n * 512 : (n + 1) * 512],
                        pss[m][:],
                        bq_t[:, n * 512 : (n + 1) * 512],
                    )
                if n == 0:
                    z_matmul(1)
                    z_recip(1)
                    bcast_norm(0)
                elif n == 1:
                    bcast_norm(1)

            # ---------------- O.T ----------------
            OT = bigp.tile([128, KB, SH], F16, tag="big", name="OT")
            for h in range(2):
                et = ets[h]
                for db in range(KB):
                    po = psatt.tile([128, 256], F32, tag="att", name=f"po_{h}_{db}")
                    for jb in range(2):
                        nc.tensor.matmul(
                            po[:],
                            qn[:, h * 2 + jb, db * 128 : (db + 1) * 128],
                            et[:, jb, :],
                            start=(jb == 0),
                            stop=(jb == 1),
                        )
                    # drain on ACT, not DVE: DVE is busy with the q-phase
                    # bias adds right when O.T starts (pso ring stalls on it)
                    nc.scalar.activation(
                        OT[:, db, h * 256 : (h + 1) * 256], po[:],
                        AF.Identity, bias=zero_col[:],
                    )
                    if db >= KB - NF8:
                        nc.vector.tensor_copy(
                            OT8[:, db - (KB - NF8), h * 256 : (h + 1) * 256],
                            po[:],
                        )

            # ---------------- final: out = concat @ Wo.T + bo ----------------
            for n in range(NO):
                last = n == NO - 1
                pss = [
                    psacc.tile([128, 512], F32, tag="acc", name=f"psf_{n}_{m}")
                    for m in range(SM)
                ]
                for kb in range(KB - NF8):
                    wt = wa.tile([128, 512], F16, tag="wa", name=f"wf_{n}_{kb}")
                    nc.sync.dma_start(wt[:], wop[n, kb][:])
                    for m in range(SM):
                        nc.tensor.matmul(
                            pss[m][:],
                            OT[:, kb, m * 128 : (m + 1) * 128],
                            wt[:],
                            start=(kb == 0),
                            stop=False,
                        )
                # last NF8 k-blocks in fp8-DoubleRow (2 k-blocks per matmul,
                # 2x rate); same x64 scale as the fp16 terms so they share the
                # accumulation group
                for pr in range(NF8 // 2):
                    wt8 = wa.tile([128, 2, 512], F8, tag="wa8", bufs=4,
                                  name=f"wf8_{n}_{pr}")
                    nc.sync.dma_start(wt8[:], wo8p[n, pr][:].bitcast(F8))
                    for m in range(SM):
                        nc.tensor.matmul(
                            pss[m][:],
                            OT8[:, 2 * pr : 2 * pr + 2, m * 128 : (m + 1) * 128],
                            wt8[:],
                            start=False,
                            stop=(pr == NF8 // 2 - 1) and not last,
                            perf_mode=DR,
                        )
                if last:
                    # fold 64*bo in on the PE (ones_row is 8.0 -> 8*8*bo)
                    for m in range(SM):
                        nc.tensor.matmul(
                            pss[m][:],
                            ones_row[:],
                            bo8_t[0:1, n * 512 : (n + 1) * 512],
                            start=False,
                            stop=True,
                        )
                for m in range(SM):
                    st = stf.tile([128, 512], F32, tag="stf", name=f"stf_{n}_{m}")
                    if last:
                        if m % 2 == 0:
                            nc.vector.tensor_scalar_mul(
                                st[:], pss[m][:], 1.0 / 64.0
                            )
                        else:
                            nc.scalar.activation(
                                st[:], pss[m][:], AF.Identity,
                                bias=zero_col[:], scale=1.0 / 64.0,
                            )
                    else:
                        nc.scalar.activation(
                            st[:], pss[m][:], AF.Identity,
                            bias=zero_col[:], scale=1.0 / 64.0,
                        )
                        nc.vector.tensor_add(
                            st[:], st[:], bo_t[:, n * 512 : (n + 1) * 512]
                        )
                    eng = nc.sync if m % 2 == 0 else nc.scalar
                    eng.dma_start(
                        out[m * 128 : (m + 1) * 128, n * 512 : (n + 1) * 512],
                        st[:],
                    )

    nc.compile()
    return nc


_NC_CACHE = None


def _pack_wo8(WoT):
    import ml_dtypes

    e4 = ml_dtypes.float8_e4m3fn
    NF8 = 4
    tail = (WoT[(KB - NF8) * 128 :, :] * 8.0).reshape(NF8 // 2, 2, 128, NO, 512)
    return np.ascontiguousarray(
        tail.transpose(3, 0, 2, 1, 4)
    ).astype(e4).view(np.uint8)


def _pack_inputs(x, Wq, bq, Wk, bk, Wv, bv, Wo, bo):
    f32 = lambda a: np.ascontiguousarray(np.asarray(a, dtype=np.float32))
    x, Wq, bq, Wk, bk, Wv, bv, Wo, bo = map(
        f32, (x, Wq, bq, Wk, bk, Wv, bv, Wo, bo)
    )
    h = np.float16
    WqT = np.ascontiguousarray(Wq.T)
    WoT = np.ascontiguousarray(Wo.T)
    shared = {
        "wqp": np.ascontiguousarray(
            WqT.reshape(KB, 128, NO, 512).transpose(2, 0, 1, 3)
        ).astype(h),
        "wkp": np.ascontiguousarray(
            Wk.reshape(KB, 128, KB, 128).transpose(0, 3, 2, 1)
        ).astype(h),
        "wvp": np.ascontiguousarray(
            Wv.reshape(KB, 128, KB, 128).transpose(0, 3, 2, 1)
        ).astype(h),
        "wop": np.ascontiguousarray(
            (WoT * 8.0).reshape(KB, 128, NO, 512).transpose(2, 0, 1, 3)
        ).astype(h),
        "wo8p": _pack_wo8(WoT),
        "bob8": np.ascontiguousarray((bo * 8.0).reshape(1, D)).astype(h),
        "bqb": np.ascontiguousarray(
            np.broadcast_to(bq.reshape(1, D), (128, D))
        ).astype(h),
        "bk_p": np.ascontiguousarray(bk.reshape(KB, 128).T),
        "bv_p": np.ascontiguousarray(bv.reshape(KB, 128).T),
        "bob": np.ascontiguousarray(
            np.broadcast_to(bo.reshape(1, D), (128, D))
        ).astype(h),
        "ones_c": np.ones((128, 1), h),
        "ones_r": np.full((1, 128), 8.0, h),
        "zero_c": np.zeros((128, 1), np.float32),
    }
    in_maps = []
    for c in range(NCORE):
        xs = x[c * SH : (c + 1) * SH]
        xTp_f = np.ascontiguousarray(
            xs.T.reshape(KB, 128, SH).transpose(1, 0, 2)
        )
        in_maps.append({"xTp": xTp_f.astype(h), **shared})
    return in_maps


def run(inputs: dict, trace: bool = False, tmpdir=None):
    """Build (cached), run on 8 cores, return (full_output, BassKernelResults)."""
    global _NC_CACHE
    in_maps = _pack_inputs(**inputs)
    if _NC_CACHE is None:
        _NC_CACHE = _build()
    res = run_bass_kernel_spmd(
        _NC_CACHE, in_maps, list(range(NCORE)), trace=trace, tmpdir=tmpdir
    )
    full = np.concatenate(
        [res.results[c]["out"] for c in range(NCORE)], axis=0
    )
    return full, res


def kernel(x, Wq, bq, Wk, bk, Wv, bv, Wo, bo):
    full, _ = run(
        dict(x=x, Wq=Wq, bq=bq, Wk=Wk, bk=bk, Wv=Wv, bv=bv, Wo=Wo, bo=bo)
    )
    return full
